# revision 7
# baseline (speedup 1.0000x reference)
"""DenseContrastiveLoss forward on 8 Trainium2 NeuronCores — v4.

Reference math:
    C = concat([f1.reshape(B,-1), f2.reshape(B,-1)])          # (512, 65536)
    G = C @ C.T ; sq[i] = ||C_i||^2
    A[i,j] = -0.01*(sq[i] + sq[j] - 2 G[i,j])
    loss = mean_i -(A[i,p(i)] - max_j A[i,j]
                    - log(sum_j exp(A-max)*offdiag + 1e-10))

Sharding: K-parallel (core c holds an fp8-e4m3 shard of C.T, pre-scaled by
1/sqrt(8) so PSUM natively accumulates G_c/8). Partial grams are reduced
across cores with peer-to-peer SDMA (remote_dma_broadcast), one 64KiB fp8
block per XOR-delta, all 7 prepped early and fired by a SINGLE trigger_dma
(this runtime wedges on a second trigger).

v4 structural changes vs the 94us v3 baseline:
- ONE input copy: both matmul operands read the same SBUF tensor whose
  512-col axis is block-XOR-permuted by a=c>>1 (position p = true block
  p^a). With identical row/col permutations the position-space gram H
  stays symmetric, so only the upper triangle (10/16 blocks) is computed
  on the PE; the 6 lower blocks are PE transposes of cast fp8 uppers.
  Halves the HBM->SBUF stream (8 MiB -> 4 MiB) and cuts PE time ~1.6x.
- Receiver-order sends: the fp8 cast of row-block e writes its col-blocks
  XOR(e)-permuted. Algebra: sender s position-e block = receiver (s^2e)'s
  own rows; true col block q^a_s lands at receiver position q^e. So every
  received slot (and the local block) is already in the receiver's own
  column order — the tree-sum needs no un-permutation, and the fp8-gram
  diagonal (host-subtracted to keep rowmax exact) always sits in block 0.
- Single trigger: 7 broadcast preps hoisted right after tile init (desc-gen
  on the idle Pool engine overlaps the DMA/matmul phase); tiny gpsimd probe
  reads of the four row tiles order the one trigger after the last cast.

Rank-independent SPMD: rdests use relative XOR addressing; the probed
logical->physical map M=[0,1,2,3,6,7,4,5] is XOR-linear so a single NEFF
serves all cores; per-core data (permutations, masks, thresholds) rides in
the in_maps.
"""

import sys

if "/opt/trn_rl_repo" not in sys.path:
    sys.path.insert(0, "/opt/trn_rl_repo")

import ml_dtypes
import numpy as np

import concourse.bass as bass  # noqa: F401
import concourse.mybir as mybir
import concourse.tile as tile
from concourse import bacc, library_config
from concourse.bass import ts
from concourse.bass_utils import run_bass_kernel_spmd

N_CORES = 8
B = 256
N = 2 * B  # 512 contrast rows
K = 65536  # feature dim (256*16*16)
P = 128
MB = N // P  # 4 row/col blocks
TEMP = 0.01
SCALE = 1.0 / np.sqrt(8.0)  # ct pre-scale: PSUM holds G/8
LSC = 2.0 * TEMP * 8.0  # logit scale in u = G/8 space (0.16)

# logical XOR-delta -> physical tpb delta for rdests (XOR-linear, probed).
XLAT = [0, 1, 2, 3, 6, 7, 4, 5]

USE_F8_T = False  # fp8 PE transposes need stride-2 PSUM out; use f32 staging


def build_nc(kshard=K // N_CORES, n_cores=N_CORES):
    nc = bacc.Bacc(
        "TRN2",
        target_bir_lowering=False,
        debug=False,
        enable_asserts=False,
        num_devices=n_cores,
    )
    NCH = kshard // P  # 64 k-chunks of 128
    ct_h = nc.dram_tensor("ct", [P, NCH, N], mybir.dt.float8e4, kind="ExternalInput")
    dsub_h = nc.dram_tensor("dsub", [P, MB * P], mybir.dt.float32, kind="ExternalInput")
    sqb_h = nc.dram_tensor("sqb", [P, N], mybir.dt.float32, kind="ExternalInput")
    adm_h = nc.dram_tensor("adm", [P, N], mybir.dt.float32, kind="ExternalInput")
    pm_h = nc.dram_tensor("pm", [P, N], mybir.dt.float32, kind="ExternalInput")
    rmask_h = nc.dram_tensor("rmask", [P, 1], mybir.dt.float32, kind="ExternalInput")
    idt = mybir.dt.float8e4 if USE_F8_T else mybir.dt.float32
    id_h = nc.dram_tensor("ident", [P, P], idt, kind="ExternalInput")
    thr_h = nc.dram_tensor("thr", [1, 2], mybir.dt.int32, kind="ExternalInput")
    out_h = nc.dram_tensor("out", [1, 1], mybir.dt.float32, kind="ExternalOutput")
    aps = dict(
        ct=ct_h.ap(), dsub=dsub_h.ap(), sqb=sqb_h.ap(), adm=adm_h.ap(),
        pm=pm_h.ap(), rmask=rmask_h.ap(), ident=id_h.ap(), thr=thr_h.ap(),
        out=out_h.ap(),
    )
    with tile.TileContext(nc) as tc:
        _body(tc, nc, aps, kshard)
    nc.compile()
    return nc


def _body(tc, nc, aps, kshard):
    ct, dsub, sqb, adm, pm = aps["ct"], aps["dsub"], aps["sqb"], aps["adm"], aps["pm"]
    rmask, ident, thr, out = aps["rmask"], aps["ident"], aps["thr"], aps["out"]
    f32 = mybir.dt.float32
    i32 = mybir.dt.int32
    f8 = mybir.dt.float8e4
    X = mybir.AxisListType.X
    add = mybir.AluOpType.add
    mult = mybir.AluOpType.mult
    sub = mybir.AluOpType.subtract
    AF = mybir.ActivationFunctionType
    DR = mybir.MatmulPerfMode.DoubleRow
    NCH = kshard // P
    groups = [2, 6] + [8] * ((NCH - 8) // 8)
    assert sum(groups) == NCH

    with (
        tc.tile_pool(name="gacc", bufs=1, space="PSUM") as gacc,
        tc.tile_pool(name="sb", bufs=1) as sb,
    ):
        nc.gpsimd.load_library(library_config.remote_dma)
        rsem = nc.alloc_semaphore("rdma_rsem")
        lsem = nc.alloc_semaphore("rdma_lsem")
        vsem = nc.alloc_semaphore("rdma_vsem")
        # register-loaded thresholds: remote increments are invisible to the
        # single-core scheduling sim, so data-loaded waits dodge its deadlock
        # detector; hardware waits are exact.
        thr_sb = sb.tile([1, 2], i32, tag="thr")
        nc.scalar.dma_start(thr_sb[:], thr)
        r_arr = nc.gpsimd.alloc_register()
        r_drn = nc.gpsimd.alloc_register()
        nc.gpsimd.load(r_arr, thr_sb[:, 0:1])
        nc.gpsimd.load(r_drn, thr_sb[:, 1:2])

        # preload both activation tables on the idle scalar engine
        dumm = sb.tile([1, 1], f32, tag="dumm")
        nc.vector.memset(dumm[:], 1.0)
        nc.scalar.activation(dumm[:], dumm[:], AF.Exp)
        nc.scalar.activation(dumm[:], dumm[:], AF.Ln)

        # ---- tiles
        acc = [gacc.tile([P, N], f32, tag=f"acc{m}", name=f"acc{m}") for m in range(MB)]
        cta = sb.tile([P, NCH, N], f8, tag="cta")
        rcv = sb.tile([P, 8, N], f8, tag="rcv")  # slots 0..6 deltas 1..7; 7 own row0
        g = [None] + [sb.tile([P, N], f8, tag=f"g{m}", name=f"g{m}") for m in range(1, MB)]
        idn = sb.tile([P, P], f8 if USE_F8_T else f32, tag="idn")
        nc.scalar.dma_start(idn[:], ident)
        # zero-init: rcv slots (late-arrival safety) + rows read by hoisted preps
        nc.vector.memset(rcv[:], 0.0)
        for m in range(1, MB):
            nc.vector.memset(g[m][:], 0.0)

        # ---- hoisted remote-send descriptor preps (desc-gen overlaps DMA/PE;
        # the data is read only at trigger time)
        for dl in range(1, 8):
            e = dl >> 1
            src = rcv[:, 7, :] if e == 0 else g[e][:]
            d_phys = XLAT[dl]
            rdests = [None] * 8
            rdests[d_phys] = (0, d_phys)
            nc.gpsimd.remote_dma_broadcast(
                rcv[:, dl - 1, :], src, rsem, lsem, rdests=rdests
            )

        # ---- input stream (single copy, both matmul operands)
        o = 0
        for gsz in groups:
            nc.sync.dma_start(cta[:, o : o + gsz, :], ct[:, o : o + gsz, :])
            o += gsz

        # ---- epilogue inputs (land during the matmul phase)
        dsub_sb = sb.tile([P, MB * P], f32, tag="dsub")
        sqb_sb = sb.tile([P, N], f32, tag="sqb")
        adm_sb = sb.tile([P, N], f32, tag="adm")
        pm_sb = sb.tile([P, N], f32, tag="pm")
        rm_sb = sb.tile([P, 1], f32, tag="rm")
        nc.scalar.dma_start(dsub_sb[:], dsub)
        nc.scalar.dma_start(sqb_sb[:], sqb)
        nc.scalar.dma_start(adm_sb[:], adm)
        nc.scalar.dma_start(pm_sb[:], pm)
        nc.scalar.dma_start(rm_sb[:], rmask)
        epsb = sb.tile([P, 1], f32, tag="epsb")
        nc.vector.memset(epsb[:], 1.0e-10)

        # ---- upper-triangle matmuls (fp8 DoubleRow, K=256/mm)
        for cc in range(0, NCH, 2):
            st = cc == 0
            sp = cc == NCH - 2
            for m in range(MB):
                nc.tensor.matmul(
                    acc[m][:, P * m : N],
                    lhsT=cta[:, cc : cc + 2, ts(m, P)],
                    rhs=cta[:, cc : cc + 2, P * m : N],
                    perf_mode=DR,
                    start=st,
                    stop=sp,
                )

        # ---- casts (receiver-order: block (e,q) -> position q^e) and
        # transpose fills for the lower triangle
        def row_dst(m):
            return rcv[:, 7, :] if m == 0 else g[m][:]

        pts = [
            gacc.tile([P, P], f8 if USE_F8_T else f32, tag=f"pt{i}", name=f"pt{i}")
            for i in range(2)
        ]
        if not USE_F8_T:
            stg = [sb.tile([P, P], f32, tag=f"stg{i}", name=f"stg{i}") for i in range(2)]
        ti = 0
        for m in range(MB):
            dst = row_dst(m)
            # diag block (m,m) -> position 0, with host fp8-diagonal subtract
            nc.vector.tensor_tensor(
                dst[:, ts(0, P)], acc[m][:, ts(m, P)], dsub_sb[:, ts(m, P)], sub
            )
            # computed upper blocks (m,q), q>m -> position q^m
            if m == 0:
                nc.vector.tensor_copy(dst[:, P:N], acc[0][:, P:N])
            else:
                for q in range(m + 1, MB):
                    nc.vector.tensor_copy(
                        dst[:, ts(q ^ m, P)], acc[m][:, ts(q, P)]
                    )
            # lower blocks (m,q), q<m = transpose of (q,m), which row q stored
            # at position m^q; destination position q^m is the same index.
            for q in range(m):
                pos = m ^ q
                pt = pts[ti % 2]
                if USE_F8_T:
                    nc.tensor.transpose(pt[:], row_dst(q)[:, ts(pos, P)], idn[:])
                else:
                    sg = stg[ti % 2]
                    nc.vector.tensor_copy(sg[:], acc[q][:, ts(m, P)])
                    nc.tensor.transpose(pt[:], sg[:], idn[:])
                nc.vector.tensor_copy(dst[:, ts(pos, P)], pt[:])
                ti += 1

        # ---- order the single trigger after every row's final cast
        prb = sb.tile([1, N], f32, tag="prb")
        nc.gpsimd.tensor_scalar_mul(prb[:], rcv[0:1, 7, :], 1.0)
        for m in range(1, MB):
            nc.gpsimd.tensor_scalar_mul(prb[:], g[m][0:1, :], 1.0)
        nc.gpsimd.trigger_dma(count=None)

        # ---- wait for all 7 arrivals (2 lanes each -> 14), release vector
        nc.gpsimd.wait_ge(rsem, r_arr)
        nc.gpsimd.sem_inc(vsem, 1)
        nc.vector.wait_ge(vsem, 1)
        s1 = sb.tile([P, 4, N], f32, tag="s1")
        nc.vector.tensor_tensor(s1[:], rcv[:, 0:4, :], rcv[:, 4:8, :], add)
        s2 = sb.tile([P, 2, N], f32, tag="s2")
        nc.vector.tensor_tensor(s2[:], s1[:, 0:2, :], s1[:, 2:4, :], add)
        s3 = sb.tile([P, N], f32, tag="s3")
        nc.vector.tensor_tensor(s3[:], s2[:, 0, :], s2[:, 1, :], add)

        # ---- softmax-loss rows on this core's 128-row block
        tt = sb.tile([P, N], f32, tag="tt")
        nc.vector.tensor_tensor(tt[:], s3[:], sqb_sb[:], add)
        mx = sb.tile([P, 1], f32, tag="mx")
        nc.vector.reduce_max(mx[:], tt[:], axis=X)
        nmx = sb.tile([P, 1], f32, tag="nmx")
        nc.vector.tensor_scalar_mul(nmx[:], mx[:], -LSC)
        tt2 = sb.tile([P, N], f32, tag="tt2")
        nc.vector.tensor_tensor(tt2[:], tt[:], adm_sb[:], add)
        ee = sb.tile([P, N], f32, tag="ee")
        sums = sb.tile([P, 1], f32, tag="sums")
        nc.scalar.activation(
            ee[:], tt2[:], AF.Exp, bias=nmx[:], scale=LSC, accum_out=sums[:]
        )
        tp_ = sb.tile([P, N], f32, tag="tp")
        nc.vector.tensor_tensor(tp_[:], tt[:], pm_sb[:], mult)
        spos = sb.tile([P, 1], f32, tag="spos")
        nc.vector.reduce_sum(spos[:], tp_[:], axis=X)
        logt = sb.tile([P, 1], f32, tag="logt")
        nc.scalar.activation(logt[:], sums[:], AF.Ln, bias=epsb[:])
        u = sb.tile([P, 1], f32, tag="u")
        nc.vector.tensor_tensor(u[:], mx[:], spos[:], sub)
        u2 = sb.tile([P, 1], f32, tag="u2")
        nc.vector.tensor_scalar_mul(u2[:], u[:], LSC)
        lrow = sb.tile([P, 1], f32, tag="lrow")
        nc.vector.tensor_tensor(lrow[:], u2[:], logt[:], add)
        # partition-reduce own 64 rows to one scalar on the now-idle PE
        lsum = gacc.tile([1, 1], f32, tag="lsum")
        nc.tensor.matmul(lsum[:], lhsT=lrow[:], rhs=rm_sb[:], start=True, stop=True)
        lout = sb.tile([1, 1], f32, tag="lout")
        nc.vector.tensor_scalar_mul(lout[:], lsum[:], 1.0)
        nc.sync.dma_start(out, lout[:])
        # sender-side drain: all 7 sends complete before teardown
        nc.gpsimd.wait_ge(lsem, r_drn)


_NC_CACHE = {}


def _get_nc():
    if "nc" not in _NC_CACHE:
        _NC_CACHE["nc"] = build_nc()
    return _NC_CACHE["nc"]


def make_in_maps(feature1, feature2, n_cores=N_CORES):
    f1 = np.asarray(feature1, dtype=np.float32).reshape(B, -1)
    f2 = np.asarray(feature2, dtype=np.float32).reshape(B, -1)
    contrast = np.concatenate([f1, f2], axis=0)  # (512, K)
    ktot = contrast.shape[1]
    kshard = ktot // n_cores
    sq = np.einsum("ij,ij->i", contrast, contrast, dtype=np.float32)  # (512,)
    ct_f8 = (contrast.T * SCALE).astype(ml_dtypes.float8_e4m3fn)  # (K, 512)
    identm = np.eye(P, dtype=ml_dtypes.float8_e4m3fn if USE_F8_T else np.float32)
    in_maps = []
    idx = np.arange(N)
    for c in range(n_cores):
        a = c >> 1
        # canonical swizzled shard: (partition, k-chunk, col), true col order
        sh = np.ascontiguousarray(
            ct_f8[c * kshard : (c + 1) * kshard].reshape(-1, P, N).transpose(1, 0, 2)
        )
        # fp8 row norms in TRUE col index space
        shf = sh.astype(np.float32)
        sq8 = np.einsum("pcj,pcj->j", shf, shf, dtype=np.float32)  # (512,)
        # block-XOR column permutation: position block p holds true block p^a
        ctp = np.empty_like(sh)
        for p in range(MB):
            ctp[:, :, P * p : P * (p + 1)] = sh[:, :, P * (p ^ a) : P * ((p ^ a) + 1)]
        # diag subtrahend: acc[e][p, 128e+p] holds true diag of row 128(e^a)+p
        dsub4 = np.zeros((P, MB * P), np.float32)
        for e in range(MB):
            dsub4[np.arange(P), P * e + np.arange(P)] = sq8[P * (e ^ a) + np.arange(P)]
        # epilogue inputs for true rows 128a..128a+127, cols in position order
        rows = P * a + np.arange(P)
        sqbc = np.tile((-0.5 * sq)[None, :], (P, 1)).astype(np.float32)
        sqbc[np.arange(P), rows] += sq[rows]
        sqbc *= 0.125
        admm = np.zeros((P, N), np.float32)
        admm[np.arange(P), rows] = -1.0e30
        pmask = np.zeros((P, N), np.float32)
        pmask[np.arange(P), (rows + B) % N] = 1.0
        # permute columns: position j <- true col cperm[j]
        cperm = np.empty(N, np.int64)
        for p in range(MB):
            cperm[P * p : P * (p + 1)] = P * (p ^ a) + np.arange(P)
        sqbc = np.ascontiguousarray(sqbc[:, cperm])
        admm = np.ascontiguousarray(admm[:, cperm])
        pmask = np.ascontiguousarray(pmask[:, cperm])
        rmv = ((np.arange(P) // 64) == (c & 1)).astype(np.float32).reshape(P, 1)
        thrv = np.array([[14, 112]], np.int32)  # 7 arrivals x2, 7 sends x16
        in_maps.append({
            "ct": ctp, "dsub": dsub4, "sqb": sqbc, "adm": admm, "pm": pmask,
            "rmask": rmv, "ident": identm, "thr": thrv,
        })
    return in_maps


def run(feature1, feature2, **spmd_kwargs):
    """Returns (loss_scalar, BassKernelResults)."""
    in_maps = make_in_maps(feature1, feature2)
    nc = _get_nc()
    res = run_bass_kernel_spmd(nc, in_maps, core_ids=list(range(N_CORES)), **spmd_kwargs)
    val = np.float32(
        sum(float(np.asarray(res.results[c]["out"]).sum(dtype=np.float64)) for c in range(N_CORES)) / N
    )
    return np.asarray(val, dtype=np.float32).reshape(()), res


def kernel(feature1, feature2):
    val, _ = run(feature1, feature2)
    return val
